# revision 21
# baseline (speedup 1.0000x reference)
"""Trainium2 Bass kernel for batched bilinear attention (sparse_attention).

Reference semantics (per batch b):
    hs_proj = hs @ W_a                      # [S, K]
    score[t,s] = ht[t,:] . hs_proj[s,:]     # = (ht @ W_a^T ... contraction over K)
    score -= rowmax(score)
    lens_b = count(source[b] != 0)
    e = exp(score) * (arange(S) < lens_b)
    a = e / rowsum(e)
    c = a @ hs
    out = tanh(concat([c, ht], -1) @ W_c + b)

Distribution: data-parallel over batch. B=16 across 8 cores -> 2 batches/core.
No collectives needed.

Per-core compute layout (per batch, T=S=H=O=1024, tiles of 128, chunks of 512):
    P[h, t]     = sum_k W_a[h,k] htT[k,t]        lhsT = W_aT tile, rhs = htT
    score[t, s] = sum_h P[h,t] hsT[h,s] + 1[t] * logmask[s]   (K=1 matmul adds mask)
    softmax over free dim s (rowmax via DVE, exp+rowsum via ACT accum, scale by 1/Z)
    aT[s, t]    = PE-transpose of a[t, s]
    cT[h, t]    = sum_s hs[s,h] aT[s,t]
    out[t, o]   = tanh( sum_h cT[h,t] Wc_top[h,o] + sum_h htT[h,t] Wc_bot[h,o]
                        + 1[t] * bias[o] )

Score path (W_aT, htT, hsT, P) is stored fp32 and matmul'd as float32r
(full PE rate at N=512); post-softmax path (hs, aT, cT, Wc_top) is bf16.
"""

import os
from contextlib import ExitStack

import ml_dtypes
import numpy as np

import concourse.bass as bass
import concourse.tile as tile
from concourse import bacc, mybir
from concourse.bass_utils import run_bass_kernel_spmd
from concourse.masks import make_identity

# ---- problem constants (hardcoded per contract) ----
B, T, S, H, O = 16, 1024, 1024, 1024, 1024
NCORES = 8
BPC = B // NCORES  # batches per core
P = 128            # partition tile
NT = T // P        # 8 tiles per 1024 dim
CHUNK = 512        # free-dim chunk (one PSUM bank of fp32)
NCH = T // CHUNK   # 2 t-chunks per batch
NEG_BIG = -1e30

F32 = mybir.dt.float32
BF16 = mybir.dt.bfloat16
BF16_NP = ml_dtypes.bfloat16

# dtype knobs.  float32r is poorly supported by walrus (producers must round
# to fp32r; known all-zero-output hazards), so all matmul inputs are bf16.
# Precision margin comes from fp32 PSUM accumulation everywhere + optional
# hi/lo compensation (COMPENSATE_*) that cancels the bf16 input-rounding error
# of selected score-path operands at ~1/8 extra PE cost each.
SDT = BF16
SDT_NP = BF16_NP
COMPENSATE_HT = False   # split htT into hi+lo bf16 in the P-projection matmul
COMPENSATE_HST = False  # split hsT into hi+lo bf16 in the score matmul

_NC_CACHE = None
LAST_RESULT = None


def _mm(ap):
    return ap


def _build_kernel(ctx: ExitStack, tc: tile.TileContext, d):
    nc = tc.nc

    # ---------------- pools ----------------
    w_pool = ctx.enter_context(tc.tile_pool(name="weights", bufs=1))
    const_pool = ctx.enter_context(tc.tile_pool(name="consts", bufs=1))
    hs_pool = ctx.enter_context(tc.tile_pool(name="hs", bufs=1))
    hsT_pool = ctx.enter_context(tc.tile_pool(name="hsT", bufs=1))
    htT_pool = ctx.enter_context(tc.tile_pool(name="htT", bufs=2))
    p_pool = ctx.enter_context(tc.tile_pool(name="psb", bufs=1))
    aT_pool = ctx.enter_context(tc.tile_pool(name="aT", bufs=1))
    cT_pool = ctx.enter_context(tc.tile_pool(name="cT", bufs=1))
    e_pool = ctx.enter_context(tc.tile_pool(name="e", bufs=2))
    a_pool = ctx.enter_context(tc.tile_pool(name="a", bufs=2))
    stat_pool = ctx.enter_context(tc.tile_pool(name="stats", bufs=2))
    out_pool = ctx.enter_context(tc.tile_pool(name="outsb", bufs=3))

    pps_pool = ctx.enter_context(tc.tile_pool(name="pps", bufs=2, space="PSUM"))
    sps_pool = ctx.enter_context(tc.tile_pool(name="sps", bufs=3, space="PSUM"))
    tp_pool = ctx.enter_context(tc.tile_pool(name="tp", bufs=2, space="PSUM"))
    mm2_pool = ctx.enter_context(tc.tile_pool(name="mm2", bufs=1, space="PSUM"))

    # ---------------- persistent weights / constants ----------------
    waT_sb = w_pool.tile([P, NT, H], BF16, tag="waT")      # [k_in, kt, h]
    nc.sync.dma_start(waT_sb[:], d["waT"].ap().rearrange("(kt p) h -> p kt h", p=P))
    wcTop_sb = w_pool.tile([P, NT, O], BF16, tag="wcTop")  # [h_in, ht, o]
    nc.sync.dma_start(wcTop_sb[:], d["wcTop"].ap().rearrange("(ht p) o -> p ht o", p=P))
    wcBot_sb = w_pool.tile([P, NT, O], BF16, tag="wcBot")
    nc.sync.dma_start(wcBot_sb[:], d["wcBot"].ap().rearrange("(ht p) o -> p ht o", p=P))

    bias_sb = const_pool.tile([1, O], BF16, tag="bias")
    nc.sync.dma_start(bias_sb[:], d["bias"].ap())
    lm_sb = const_pool.tile([1, BPC, S], BF16, tag="lm")
    nc.sync.dma_start(lm_sb[:], d["lm"].ap().rearrange("(x b) s -> x b s", x=1))
    ones_sb = const_pool.tile([1, P], BF16, tag="ones")
    nc.vector.memset(ones_sb[:], 1.0)
    ident_sb = const_pool.tile([P, P], BF16, tag="ident")
    make_identity(nc, ident_sb[:])

    # ---------------- per-batch program ----------------
    for b in range(BPC):
        hs_sb = hs_pool.tile([P, NT, H], BF16, tag="hs")   # [s_in, st, h]
        nc.sync.dma_start(hs_sb[:], d["hs"].ap()[b].rearrange("(st p) h -> p st h", p=P))
        hsT_sb = hsT_pool.tile([P, NT, S], SDT, tag="hsT")  # [h_in, ht, s]
        nc.sync.dma_start(hsT_sb[:], d["hsT"].ap()[b].rearrange("(ht p) s -> p ht s", p=P))
        if COMPENSATE_HST:
            hsLoT_sb = hsT_pool.tile([P, NT, S], BF16, tag="hsLoT")
            nc.sync.dma_start(hsLoT_sb[:], d["hsLoT"].ap()[b].rearrange("(ht p) s -> p ht s", p=P))

        for ch in range(NCH):
            tlo = ch * CHUNK  # global t offset of this chunk

            htT_sb = htT_pool.tile([P, NT, CHUNK], BF16, tag="htT")  # [k_in, kt, t]
            nc.sync.dma_start(
                htT_sb[:],
                d["htT"].ap()[b].rearrange("(kt p) t -> p kt t", p=P)[:, :, tlo : tlo + CHUNK],
            )
            if COMPENSATE_HT:
                htLo_sb = htT_pool.tile([P, NT, CHUNK], BF16, tag="htLo")
                nc.sync.dma_start(
                    htLo_sb[:],
                    d["htLo"].ap()[b].rearrange("(kt p) t -> p kt t", p=P)[:, :, tlo : tlo + CHUNK],
                )

            # ---- P[h, t-chunk] = W_a @ htT ----
            p_sb = p_pool.tile([P, NT, CHUNK], SDT, tag="psb")  # [h_in, ht, t]
            for hh in range(NT):
                pps = pps_pool.tile([P, CHUNK], F32, tag="pps")
                for kt in range(NT):
                    nc.tensor.matmul(
                        pps[:],
                        waT_sb[:, kt, hh * P : (hh + 1) * P],
                        htT_sb[:, kt, :],
                        start=(kt == 0),
                        stop=(kt == NT - 1) and not COMPENSATE_HT,
                    )
                if COMPENSATE_HT:
                    for kt in range(NT):
                        nc.tensor.matmul(
                            pps[:],
                            waT_sb[:, kt, hh * P : (hh + 1) * P],
                            htLo_sb[:, kt, :],
                            start=False,
                            stop=(kt == NT - 1),
                        )
                nc.vector.tensor_copy(p_sb[:, hh, :], pps[:])

            # ---- per t-tile: score + softmax + transpose ----
            aT_sb = aT_pool.tile([P, NT, CHUNK], BF16, tag="aT")  # [s_in, st, t]
            for tl in range(NCH * 2):  # 4 t-tiles of 128 in the 512 chunk
                tsl = slice(tl * P, (tl + 1) * P)

                sps_list = []
                for sc in range(S // CHUNK):
                    ssl = slice(sc * CHUNK, (sc + 1) * CHUNK)
                    sps = sps_pool.tile([P, CHUNK], F32, tag="sps")
                    for hh in range(NT):
                        nc.tensor.matmul(
                            sps[:],
                            p_sb[:, hh, tsl],
                            hsT_sb[:, hh, ssl],
                            start=(hh == 0),
                            stop=False,
                        )
                    if COMPENSATE_HST:
                        for hh in range(NT):
                            nc.tensor.matmul(
                                sps[:],
                                p_sb[:, hh, tsl],
                                hsLoT_sb[:, hh, ssl],
                                start=False,
                                stop=False,
                            )
                    # add log-mask row: score += ones[t] * lm[s]
                    nc.tensor.matmul(
                        sps[:],
                        ones_sb[:, :],
                        lm_sb[:, b, ssl],
                        start=False,
                        stop=True,
                    )
                    sps_list.append(sps)

                # softmax over s (free dim), chunked.  One stat tile per t-tile:
                # cols 0:m0 1:m1 2:negm 3:z0 4:z1 5:rz
                st_t = stat_pool.tile([P, 6], F32, tag="stat")
                nc.vector.tensor_reduce(st_t[:, 0:1], sps_list[0][:], axis=mybir.AxisListType.X, op=mybir.AluOpType.max)
                nc.vector.tensor_reduce(st_t[:, 1:2], sps_list[1][:], axis=mybir.AxisListType.X, op=mybir.AluOpType.max)
                nc.vector.tensor_tensor(st_t[:, 2:3], st_t[:, 0:1], st_t[:, 1:2], op=mybir.AluOpType.max)
                nc.vector.tensor_scalar_mul(st_t[:, 2:3], st_t[:, 2:3], -1.0)

                e_sb = e_pool.tile([P, S], F32, tag="e")
                nc.scalar.activation(
                    e_sb[:, 0:CHUNK], sps_list[0][:], mybir.ActivationFunctionType.Exp,
                    bias=st_t[:, 2:3], scale=1.0, accum_out=st_t[:, 3:4],
                )
                nc.scalar.activation(
                    e_sb[:, CHUNK:S], sps_list[1][:], mybir.ActivationFunctionType.Exp,
                    bias=st_t[:, 2:3], scale=1.0, accum_out=st_t[:, 4:5],
                )
                nc.vector.tensor_tensor(st_t[:, 5:6], st_t[:, 3:4], st_t[:, 4:5], op=mybir.AluOpType.add)
                nc.vector.reciprocal(st_t[:, 5:6], st_t[:, 5:6])

                a_sb = a_pool.tile([P, S], BF16, tag="a")
                nc.vector.tensor_scalar_mul(a_sb[:], e_sb[:], st_t[:, 5:6])

                # aT[s, t-tile] via PE transpose of each [128t x 128s] block
                for st in range(NT):
                    tp = tp_pool.tile([P, P], BF16, tag="tp")
                    nc.tensor.transpose(tp[:], a_sb[:, st * P : (st + 1) * P], ident_sb[:])
                    nc.scalar.copy(aT_sb[:, st, tsl], tp[:])

            # ---- cT[h, t-chunk] = hs @ aT ----
            cT_sb = cT_pool.tile([P, NT, CHUNK], BF16, tag="cT")  # [h_in, ht, t]
            for hh in range(NT):
                cps = mm2_pool.tile([P, CHUNK], F32, tag="mm2")
                for st in range(NT):
                    nc.tensor.matmul(
                        cps[:],
                        hs_sb[:, st, hh * P : (hh + 1) * P],
                        aT_sb[:, st, :],
                        start=(st == 0),
                        stop=(st == NT - 1),
                    )
                nc.vector.tensor_copy(cT_sb[:, hh, :], cps[:])

            # ---- out[t, o] = tanh(cT.T @ WcTop + htT.T @ WcBot + bias) ----
            for tl in range(NCH * 2):
                tsl = slice(tl * P, (tl + 1) * P)
                for oc in range(O // CHUNK):
                    osl = slice(oc * CHUNK, (oc + 1) * CHUNK)
                    ops = mm2_pool.tile([P, CHUNK], F32, tag="mm2")
                    for hh in range(NT):
                        nc.tensor.matmul(
                            ops[:],
                            cT_sb[:, hh, tsl],
                            wcTop_sb[:, hh, osl],
                            start=(hh == 0),
                            stop=False,
                        )
                    for hh in range(NT):
                        nc.tensor.matmul(
                            ops[:],
                            htT_sb[:, hh, tsl],
                            wcBot_sb[:, hh, osl],
                            start=False,
                            stop=False,
                        )
                    nc.tensor.matmul(
                        ops[:], ones_sb[:, :], bias_sb[:, osl],
                        start=False, stop=True,
                    )
                    out_sb = out_pool.tile([P, CHUNK], F32, tag="out")
                    nc.scalar.activation(out_sb[:], ops[:], mybir.ActivationFunctionType.Tanh)
                    nc.sync.dma_start(d["out"].ap()[b, tlo + tl * P : tlo + (tl + 1) * P, osl], out_sb[:])


def _get_nc():
    global _NC_CACHE
    if _NC_CACHE is not None:
        return _NC_CACHE

    nc = bacc.Bacc("TRN2", target_bir_lowering=False, debug=False)
    d = {
        "htT": nc.dram_tensor("htT", [BPC, H, T], BF16, kind="ExternalInput"),
        "hsT": nc.dram_tensor("hsT", [BPC, H, S], SDT, kind="ExternalInput"),
        "hs": nc.dram_tensor("hs", [BPC, S, H], BF16, kind="ExternalInput"),
        "waT": nc.dram_tensor("waT", [H, H], BF16, kind="ExternalInput"),
        "wcTop": nc.dram_tensor("wcTop", [H, O], BF16, kind="ExternalInput"),
        "wcBot": nc.dram_tensor("wcBot", [H, O], BF16, kind="ExternalInput"),
        "bias": nc.dram_tensor("bias", [1, O], BF16, kind="ExternalInput"),
        "lm": nc.dram_tensor("lm", [BPC, S], BF16, kind="ExternalInput"),
        "out": nc.dram_tensor("out", [BPC, T, O], F32, kind="ExternalOutput"),
    }
    if COMPENSATE_HT:
        d["htLo"] = nc.dram_tensor("htLo", [BPC, H, T], BF16, kind="ExternalInput")
    if COMPENSATE_HST:
        d["hsLoT"] = nc.dram_tensor("hsLoT", [BPC, H, S], BF16, kind="ExternalInput")
    with tile.TileContext(nc) as tc:
        with ExitStack() as ctx:
            _build_kernel(ctx, tc, d)
    nc.compile()
    _NC_CACHE = nc
    return nc


def kernel(ht, hs, W_a, W_c, b, source):
    global LAST_RESULT
    ht = np.asarray(ht, dtype=np.float32)
    hs = np.asarray(hs, dtype=np.float32)
    W_a = np.asarray(W_a, dtype=np.float32)
    W_c = np.asarray(W_c, dtype=np.float32)
    b = np.asarray(b, dtype=np.float32)
    source = np.asarray(source)

    # host-side layout prep (sharding + per-layout copies)
    htT_f = np.ascontiguousarray(ht.transpose(0, 2, 1))                # [B, H, T] fp32
    hsT_f = np.ascontiguousarray(hs.transpose(0, 2, 1))                # [B, H, S] fp32
    htT = htT_f.astype(BF16_NP)
    hsT = hsT_f.astype(SDT_NP)
    hs_b = hs.astype(BF16_NP)
    waT = np.ascontiguousarray(W_a.T).astype(BF16_NP)                  # [K, H]
    wcTop = np.ascontiguousarray(W_c[:H]).astype(BF16_NP)
    wcBot = np.ascontiguousarray(W_c[H:]).astype(BF16_NP)
    bias = b.reshape(1, O).astype(BF16_NP)

    lens = (source != 0).sum(axis=1)                                   # [B]
    lm = np.where(np.arange(S)[None, :] < lens[:, None], 0.0, NEG_BIG).astype(BF16_NP)

    if COMPENSATE_HT:
        htLo = (htT_f - htT.astype(np.float32)).astype(BF16_NP)
    if COMPENSATE_HST:
        hsLoT = (hsT_f - hsT.astype(np.float32)).astype(BF16_NP)

    in_maps = []
    for c in range(NCORES):
        sl = slice(c * BPC, (c + 1) * BPC)
        m = {
            "htT": htT[sl],
            "hsT": hsT[sl],
            "hs": hs_b[sl],
            "waT": waT,
            "wcTop": wcTop,
            "wcBot": wcBot,
            "bias": bias,
            "lm": lm[sl],
        }
        if COMPENSATE_HT:
            m["htLo"] = htLo[sl]
        if COMPENSATE_HST:
            m["hsLoT"] = hsLoT[sl]
        in_maps.append(m)

    nc = _get_nc()
    res = run_bass_kernel_spmd(nc, in_maps, core_ids=list(range(NCORES)))
    LAST_RESULT = res
    out = np.concatenate([r["out"] for r in res.results], axis=0)
    return np.ascontiguousarray(out.astype(np.float32))
